# revision 1
# baseline (speedup 1.0000x reference)
"""Trainium2 Bass kernel for the fuzzy joint-membership layer.

Math (derived from the reference 2-qubit circuit, verified vs oracle):
  out[b, 2p,   c] = 0.5 + 0.5*cos(theta_c)*cos(x0) - 0.5*sin(theta_c)*sin(x0)*sin(x1)
  out[b, 2p+1, c] = 0.5 + 0.5*cos(x0)*cos(x1)
where x0 = xf[b, pair_idx[b,p,0]], x1 = xf[b, pair_idx[b,p,1]].

Sharding: pure data parallel, batch 4096 -> 8 cores x 512 rows.

Device kernel per 128-row tile (5 column-split chunks, pipelined):
  - DMA in xf [128,3072] f32 and indices [128,920] i16
  - gpsimd ap_gather with the natural idx layout: each Q7 core's 16-partition
    group round-robins its 16 rows' index lists, so the useful gathered value
    for partition p (p%16 == s) lands at column 16*j + s of gout
    (5 splits of 2944 idx each: smaller calls measure ~8% faster/idx and
    overlap downstream with the serial gpsimd gathers)
  - PE extracts the diagonal: gv = sum_s diag(p%16==s) @ gout[:, s::16]
    accumulated in PSUM (partition-strided SBUF DMA is broken in this stack)
  - DVE range-reduction (magic round) + ACT Sin: cv = cos(vals), sv = -sin(vals)
  - DVE: W = sv_even*sv_odd = sin*sin, E = cv_even*cv_odd
  - per class c: out_even_c = (cos(x0)*hct_c + 0.5) + W*(-hst_c)   (2 DVE ops)
                 out_odd_c  = Copy(E*0.5 + 0.5)                    (1 ACT op)
  - DMA out [128, 9200]
"""

import math
import numpy as np

B, PIX, NPAIR, C = 4096, 3072, 460, 10
NG = 2 * NPAIR          # 920 gathered values per row
OUTW = NG * C           # 9200
NCORES = 8
BS = B // NCORES        # 512 rows per core
TILES = BS // 128       # 4

_cache = {}


def _ensure_path():
    try:
        import concourse  # noqa: F401
    except ImportError:
        import sys
        sys.path.insert(0, "/opt/trn_rl_repo")


def build_nc(bs=BS):
    _ensure_path()
    from contextlib import ExitStack
    import concourse.tile as tile
    from concourse import bacc, mybir

    f32, i16 = mybir.dt.float32, mybir.dt.int16
    Sin = mybir.ActivationFunctionType.Sin
    Copy = mybir.ActivationFunctionType.Copy
    mult = mybir.AluOpType.mult
    add = mybir.AluOpType.add
    ntiles = bs // 128

    nc = bacc.Bacc("TRN2", target_bir_lowering=False, debug=False)
    x_ext = nc.declare_dram_parameter("x", [bs, PIX], f32, isOutput=False)
    idx_ext = nc.declare_dram_parameter("idx", [bs, NG], i16, isOutput=False)
    th_ext = nc.declare_dram_parameter("theta", [128, C], f32, isOutput=False)
    mk_ext = nc.declare_dram_parameter("masks", [128, 16 * 128], f32, isOutput=False)
    out_ext = nc.declare_dram_parameter("out", [bs, OUTW], f32, isOutput=True)

    with tile.TileContext(nc) as tc, ExitStack() as ctx:
        cpool = ctx.enter_context(tc.tile_pool(name="const", bufs=1))
        xpool = ctx.enter_context(tc.tile_pool(name="xf", bufs=2))
        ipool = ctx.enter_context(tc.tile_pool(name="idx", bufs=2))
        gpool = ctx.enter_context(tc.tile_pool(name="gout", bufs=2))
        ppool = ctx.enter_context(tc.tile_pool(name="gvp", bufs=2, space="PSUM"))
        tpool = ctx.enter_context(tc.tile_pool(name="trig", bufs=2))
        wpool = ctx.enter_context(tc.tile_pool(name="we", bufs=2))
        opool = ctx.enter_context(tc.tile_pool(name="ot", bufs=2))

        masks = cpool.tile([128, 16 * 128], f32)
        nc.sync.dma_start(out=masks[:], in_=mk_ext[:, :])

        # Scalar-engine Sin only accepts [-pi, pi]. Range-reduce with the
        # round-to-nearest magic trick: n = (v/2pi + M) - M, -r = 2pi*n - v.
        # Then -sin(v) = Sin(-r) and cos(v) = Sin(pi/2 - |r|); the sin sign
        # flip cancels in sin*sin products and is absorbed into nhst.
        sub_ = mybir.AluOpType.subtract
        maxop = mybir.AluOpType.max
        PI, TWO_PI = math.pi, 2 * math.pi
        MAGIC, INV2PI = 1.5 * 2 ** 23, 1.0 / (2 * math.pi)
        pihalf = cpool.tile([128, 1], f32)
        nc.vector.memset(pihalf[:], PI / 2)
        zerob = cpool.tile([128, 1], f32)
        nc.vector.memset(zerob[:], 0.0)

        def trig(pool, src, width, tagp):
            """returns (cv, svN) = (cos(src), -sin(src)), width cols."""
            t1 = pool.tile([128, width], f32, tag=tagp + "t1")
            nc.vector.tensor_scalar(t1[:], src, INV2PI, MAGIC, mult, add)
            nc.vector.tensor_scalar(t1[:], t1[:], MAGIC, None, sub_)
            nc.vector.tensor_scalar(t1[:], t1[:], TWO_PI, None, mult)
            negr = pool.tile([128, width], f32, tag=tagp + "negr")
            nc.vector.tensor_tensor(negr[:], t1[:], src, sub_)
            nc.vector.tensor_scalar(t1[:], negr[:], -1.0, None, mult)
            nc.vector.tensor_tensor(t1[:], t1[:], negr[:], maxop)  # |r|
            cv = pool.tile([128, width], f32, tag=tagp + "cv")
            svN = pool.tile([128, width], f32, tag=tagp + "svN")
            nc.scalar.activation(svN[:], negr[:], Sin, bias=zerob[:, 0:1])
            nc.scalar.activation(cv[:], t1[:], Sin, bias=pihalf[:, 0:1], scale=-1.0)
            return cv, svN

        # theta coefficients: hct = 0.5*cos(theta), nhst = -0.5*sin(theta)
        th_sb = cpool.tile([128, C], f32)
        nc.sync.dma_start(out=th_sb[:], in_=th_ext[:, :])
        cvt, svNt = trig(cpool, th_sb[:], C, "th")
        hcoef = cpool.tile([128, 2 * C], f32)
        nc.vector.tensor_scalar(hcoef[:, 0:C], cvt[:], 0.5, None, mult)
        nc.vector.tensor_scalar(hcoef[:, C:2 * C], svNt[:], 0.5, None, mult)
        hct = hcoef[:, 0:C]        # 0.5*cos(theta)
        nhst = hcoef[:, C:2 * C]   # -0.5*sin(theta) = 0.5*svN

        for t in range(ntiles):
            rows = slice(t * 128, (t + 1) * 128)
            xf = xpool.tile([128, PIX], f32)
            nc.sync.dma_start(out=xf[:], in_=x_ext[rows, :])
            idxt = ipool.tile([128, NG], i16)
            nc.sync.dma_start(out=idxt[:], in_=idx_ext[rows, :])

            # Gather in 4 column-splits so downstream overlaps the serial
            # gpsimd gathers and the Q7 idx scratch stays small. Extract the
            # diagonal (useful value for partition p at col 16j+p%16) via PE:
            # gv = sum_s diag(p%16==s) @ gout[:, s::16], accumulated in PSUM.
            NSPLIT = 5
            HNG = NG // NSPLIT  # 184
            ot = opool.tile([128, OUTW], f32)
            for h in range(NSPLIT):
                gout = gpool.tile([128, 16 * HNG], f32, tag="gout")
                nc.gpsimd.ap_gather(
                    out_ap=gout[:],
                    in_ap=xf[:],
                    idxs_ap=idxt[:, h * HNG:(h + 1) * HNG],
                    channels=128,
                    num_elems=PIX,
                    d=1,
                    num_idxs=16 * HNG,
                )
                gvp = ppool.tile([128, HNG], f32, tag="gvp")
                for s in range(16):
                    rhs = gout[:, s: 16 * HNG: 16]
                    nc.tensor.matmul(
                        gvp[:], masks[:, s * 128:(s + 1) * 128], rhs,
                        start=(s == 0), stop=(s == 15),
                    )
                cv, sv = trig(tpool, gvp[:], HNG, "g")
                w = wpool.tile([128, HNG // 2], f32, tag="w")
                e = wpool.tile([128, HNG // 2], f32, tag="e")
                nc.vector.tensor_tensor(w[:], sv[:, 0:HNG:2], sv[:, 1:HNG:2], mult)
                nc.vector.tensor_tensor(e[:], cv[:, 0:HNG:2], cv[:, 1:HNG:2], mult)

                base = h * (OUTW // NSPLIT)
                for c in range(C):
                    ev = ot[:, base + c: base + OUTW // NSPLIT: 2 * C]
                    nc.vector.tensor_scalar(ev, cv[:, 0:HNG:2], hct[:, c:c + 1], 0.5, mult, add)
                    nc.vector.scalar_tensor_tensor(ev, w[:], nhst[:, c:c + 1], ev, mult, add)
                    ov = ot[:, base + C + c: base + OUTW // NSPLIT: 2 * C]
                    nc.scalar.activation(ov, e[:], Copy, bias=0.5, scale=0.5)
            nc.sync.dma_start(out=out_ext[rows, :], in_=ot[:])

    nc.compile()
    return nc


def _masks_np():
    if "masks" not in _cache:
        m = np.zeros((128, 16 * 128), dtype=np.float32)
        p = np.arange(128)
        m[p, (p % 16) * 128 + p] = 1.0
        _cache["masks"] = np.ascontiguousarray(m)
    return _cache["masks"]


def _get_nc():
    if "nc" not in _cache:
        _cache["nc"] = build_nc()
    return _cache["nc"]


def kernel(x, pair_idx, theta):
    _ensure_path()
    from concourse.bass_utils import run_bass_kernel_spmd

    nc = _get_nc()
    xs = np.ascontiguousarray(np.asarray(x, dtype=np.float32).reshape(B, PIX))
    idx16 = np.ascontiguousarray(
        np.asarray(pair_idx).reshape(B, NG).astype(np.int16)
    )
    thb = np.ascontiguousarray(
        np.tile(np.asarray(theta, dtype=np.float32).reshape(1, C), (128, 1))
    )
    in_maps = [
        {
            "x": xs[k * BS:(k + 1) * BS],
            "idx": idx16[k * BS:(k + 1) * BS],
            "theta": thb,
            "masks": _masks_np(),
        }
        for k in range(NCORES)
    ]
    res = run_bass_kernel_spmd(nc, in_maps, list(range(NCORES))).results
    out = np.concatenate([res[k]["out"] for k in range(NCORES)], axis=0)
    return out.reshape(B, NG, C).astype(np.float32)



# revision 7
# speedup vs baseline: 7.3324x; 7.3324x over previous
"""Trainium2 Bass kernel for the fuzzy joint-membership layer.

Math (derived from the reference 2-qubit circuit, verified vs oracle):
  out[b, 2p,   c] = 0.5 + 0.5*cos(theta_c)*cos(x0) - 0.5*sin(theta_c)*sin(x0)*sin(x1)
  out[b, 2p+1, c] = 0.5 + 0.5*cos(x0)*cos(x1)
where x0 = xf[b, pair_idx[b,p,0]], x1 = xf[b, pair_idx[b,p,1]].

Sharding: pure data parallel, batch 4096 -> 8 cores x 512 rows.

Gather strategy (v2): instead of ap_gather (whose Q7 loop costs ~24ns/idx and
serialized the whole kernel at ~83us per 2944-idx call), run the gather as a
host-inverted local_scatter:
  - host builds A_main[b, pix] = output slot of the FIRST occurrence of pixel
    `pix` in row b's 920-entry index list (slot j = pair j's x0, 460+j = x1),
    -1 if unused; and Acomb[b, slot] = slot of the NEXT occurrence of the same
    pixel (chained generations), -1 if none.
  - device: G0 = local_scatter(xh, A_main)  (one vectorized 3072-scan per tile;
    per-partition independent indices, runs in Q7 local RAM)
    Gk = local_scatter(G{k-1}, Acomb) for k=1..m-1 resolves duplicate gathers:
    generation-k slots pick their value from generation-(k-1) slots; entries of
    other generations harmlessly scatter zeros. v = sum_k Gk (disjoint support).
  - m = max index multiplicity per row (data-dependent, ~6), baked at build.
x is converted to f16 on host (payload for the 2-byte scatter; |err| ~5e-4
after trig, tolerance is 2e-2).

Device per 128-row tile:
  - DMA in xh [128,3072] f16, A_main [128,3072] i16, Acomb [128,920] i16
  - gpsimd: m local_scatters (1 x 3072-scan + (m-1) x 920-scan)
  - DVE: merge adds (f16), range-reduce via magic round, |r| via abs_max
  - ACT: vf = Copy(v) f16->f32, svN = Sin(-r) = -sin(v), cv = Sin(pi/2-|r|)
  - DVE: W = svN0*svN1 = sin*sin, E = cv0*cv1
  - per class c: out_even_c = (cos(x0)*hct_c + 0.5) + W*nhst_c   (2 DVE ops)
                 out_odd_c  = Copy(E*0.5 + 0.5)                  (1 ACT op)
  - DMA out [128, 9200]
"""

import math
import numpy as np

B, PIX, NPAIR, C = 4096, 3072, 460, 10
NG = 2 * NPAIR          # 920 gathered values per row
OUTW = NG * C           # 9200
NCORES = 8
BS = B // NCORES        # 512 rows per core
TILES = BS // 128       # 4

_cache = {}


def _ensure_path():
    try:
        import concourse  # noqa: F401
    except ImportError:
        import sys
        sys.path.insert(0, "/opt/trn_rl_repo")


def build_nc(m, bs=BS):
    """m = max per-row index multiplicity -> 1 main scatter + (m-1) copy passes."""
    _ensure_path()
    from contextlib import ExitStack
    import concourse.tile as tile
    from concourse import bacc, mybir

    f32, f16, i16 = mybir.dt.float32, mybir.dt.float16, mybir.dt.int16
    Sin = mybir.ActivationFunctionType.Sin
    Copy = mybir.ActivationFunctionType.Copy
    mult = mybir.AluOpType.mult
    add = mybir.AluOpType.add
    sub_ = mybir.AluOpType.subtract
    maxop = mybir.AluOpType.max
    ntiles = bs // 128

    nc = bacc.Bacc("TRN2", target_bir_lowering=False, debug=False)
    xh_ext = nc.declare_dram_parameter("xh", [bs, PIX], f16, isOutput=False)
    am_ext = nc.declare_dram_parameter("amain", [bs, PIX], i16, isOutput=False)
    ac_ext = nc.declare_dram_parameter("acomb", [bs, NG], i16, isOutput=False)
    th_ext = nc.declare_dram_parameter("theta", [128, C], f32, isOutput=False)
    out_ext = nc.declare_dram_parameter("out", [bs, OUTW], f32, isOutput=True)

    PI, TWO_PI = math.pi, 2 * math.pi
    MAGIC, INV2PI = 1.5 * 2 ** 23, 1.0 / (2 * math.pi)

    with tile.TileContext(nc) as tc, ExitStack() as ctx:
        cpool = ctx.enter_context(tc.tile_pool(name="const", bufs=1))
        pihalf = cpool.tile([128, 1], f32)
        nc.vector.memset(pihalf[:], PI / 2)
        zerob = cpool.tile([128, 1], f32)
        nc.vector.memset(zerob[:], 0.0)
        xpool = ctx.enter_context(tc.tile_pool(name="xh", bufs=2))
        apool = ctx.enter_context(tc.tile_pool(name="amain", bufs=2))
        bpool = ctx.enter_context(tc.tile_pool(name="acomb", bufs=2))
        gpool = ctx.enter_context(tc.tile_pool(name="gens", bufs=2))
        spool = ctx.enter_context(tc.tile_pool(name="trig", bufs=2))
        opool = ctx.enter_context(tc.tile_pool(name="ot", bufs=2))

        def trig(pool, src, width, tagp):
            """returns (cv, svN) = (cos(src), -sin(src)), width cols, f32.

            Range-reduce with round-to-nearest magic: n = (v/2pi + M) - M,
            -r = 2pi*n - v. Then -sin(v) = Sin(-r), cos(v) = Sin(pi/2 - |r|).
            """
            t1 = pool.tile([128, width], f32, tag=tagp + "t1")
            nc.vector.tensor_scalar(t1[:], src, INV2PI, MAGIC, mult, add)
            nc.vector.tensor_scalar(t1[:], t1[:], MAGIC, None, sub_)
            nc.vector.tensor_scalar(t1[:], t1[:], TWO_PI, None, mult)
            negr = pool.tile([128, width], f32, tag=tagp + "negr")
            nc.vector.tensor_tensor(negr[:], t1[:], src, sub_)
            nc.vector.tensor_scalar(t1[:], negr[:], -1.0, None, mult)
            nc.vector.tensor_tensor(t1[:], t1[:], negr[:], maxop)  # |r|
            cv = pool.tile([128, width], f32, tag=tagp + "cv")
            svN = pool.tile([128, width], f32, tag=tagp + "svN")
            nc.scalar.activation(svN[:], negr[:], Sin, bias=zerob[:, 0:1])
            nc.scalar.activation(cv[:], t1[:], Sin, bias=pihalf[:, 0:1], scale=-1.0)
            return cv, svN

        # theta coefficients: hct = 0.5*cos(theta), nhst = -0.5*sin(theta)
        th_sb = cpool.tile([128, C], f32)
        nc.sync.dma_start(out=th_sb[:], in_=th_ext[:, :])
        cvt, svNt = trig(cpool, th_sb[:], C, "th")
        hcoef = cpool.tile([128, 2 * C], f32)
        nc.vector.tensor_scalar(hcoef[:, 0:C], cvt[:], 0.5, None, mult)
        nc.vector.tensor_scalar(hcoef[:, C:2 * C], svNt[:], 0.5, None, mult)
        hct = hcoef[:, 0:C]        # 0.5*cos(theta)
        nhst = hcoef[:, C:2 * C]   # -0.5*sin(theta)

        for t in range(ntiles):
            rows = slice(t * 128, (t + 1) * 128)
            xh = xpool.tile([128, PIX], f16)
            nc.sync.dma_start(out=xh[:], in_=xh_ext[rows, :])
            am = apool.tile([128, PIX], i16)
            nc.sync.dma_start(out=am[:], in_=am_ext[rows, :])
            ac = bpool.tile([128, NG], i16)
            nc.sync.dma_start(out=ac[:], in_=ac_ext[rows, :])

            gens = [
                gpool.tile([128, NG], f16, tag=f"g{k}", name=f"g{k}")
                for k in range(m)
            ]
            nc.gpsimd.local_scatter(gens[0][:], xh[:], am[:], 128, NG, PIX)
            for k in range(1, m):
                nc.gpsimd.local_scatter(gens[k][:], gens[k - 1][:], ac[:], 128, NG, NG)

            v = spool.tile([128, NG], f16, tag="v")
            if m == 1:
                v = gens[0]
            else:
                nc.vector.tensor_tensor(v[:], gens[0][:], gens[1][:], add)
                for k in range(2, m):
                    nc.vector.tensor_tensor(v[:], v[:], gens[k][:], add)
            vf = spool.tile([128, NG], f32, tag="vf")
            nc.scalar.activation(vf[:], v[:], Copy)

            cv, svN = trig(spool, vf[:], NG, "g")
            w = spool.tile([128, NPAIR], f32, tag="w")
            e = spool.tile([128, NPAIR], f32, tag="e")
            nc.vector.tensor_tensor(w[:], svN[:, 0:NPAIR], svN[:, NPAIR:NG], mult)
            nc.vector.tensor_tensor(e[:], cv[:, 0:NPAIR], cv[:, NPAIR:NG], mult)

            ot = opool.tile([128, OUTW], f32)
            for c in range(C):
                ev = ot[:, c: OUTW: 2 * C]
                nc.vector.tensor_scalar(ev, cv[:, 0:NPAIR], hct[:, c:c + 1], 0.5, mult, add)
                nc.vector.scalar_tensor_tensor(ev, w[:], nhst[:, c:c + 1], ev, mult, add)
                ov = ot[:, C + c: OUTW: 2 * C]
                nc.scalar.activation(ov, e[:], Copy, bias=0.5, scale=0.5)
            nc.sync.dma_start(out=out_ext[rows, :], in_=ot[:])

    nc.compile()
    return nc


def _prep_indices(pair_idx):
    """Invert the per-row gather map into scatter index arrays.

    Slot layout: slot j = pair j's x0, slot 460+j = pair j's x1.
    Returns (A_main [B,3072] i16, Acomb [B,920] i16, m).
    """
    pi = np.asarray(pair_idx).reshape(B, NPAIR, 2)
    L = np.concatenate([pi[:, :, 0], pi[:, :, 1]], axis=1).astype(np.int64)
    order = np.argsort(L, axis=1, kind="stable")
    spix = np.take_along_axis(L, order, axis=1)
    first = np.ones_like(spix, dtype=bool)
    first[:, 1:] = spix[:, 1:] != spix[:, :-1]
    t = np.broadcast_to(np.arange(NG)[None, :], (B, NG))
    firstpos = np.maximum.accumulate(np.where(first, t, 0), axis=1)
    occ_sorted = t - firstpos
    m = int(occ_sorted.max()) + 1

    A_main = np.full((B, PIX), -1, np.int16)
    rr, cc = np.nonzero(occ_sorted == 0)
    A_main[rr, spix[rr, cc]] = order[rr, cc].astype(np.int16)

    Acomb = np.full((B, NG), -1, np.int16)
    nxt = np.zeros((B, NG), dtype=bool)
    nxt[:, :-1] = spix[:, 1:] == spix[:, :-1]
    rr2, cc2 = np.nonzero(nxt)
    Acomb[rr2, order[rr2, cc2]] = order[rr2, cc2 + 1].astype(np.int16)
    return A_main, Acomb, m


def _get_nc(m):
    key = ("nc", m)
    if key not in _cache:
        _cache[key] = build_nc(m)
    return _cache[key]


def kernel(x, pair_idx, theta):
    _ensure_path()
    from concourse.bass_utils import run_bass_kernel_spmd

    xh = np.ascontiguousarray(
        np.asarray(x, dtype=np.float32).reshape(B, PIX).astype(np.float16)
    )
    A_main, Acomb, m = _prep_indices(pair_idx)
    thb = np.ascontiguousarray(
        np.tile(np.asarray(theta, dtype=np.float32).reshape(1, C), (128, 1))
    )
    nc = _get_nc(m)
    in_maps = [
        {
            "xh": xh[k * BS:(k + 1) * BS],
            "amain": A_main[k * BS:(k + 1) * BS],
            "acomb": Acomb[k * BS:(k + 1) * BS],
            "theta": thb,
        }
        for k in range(NCORES)
    ]
    res = run_bass_kernel_spmd(nc, in_maps, list(range(NCORES))).results
    out = np.concatenate([res[k]["out"] for k in range(NCORES)], axis=0)
    return out.reshape(B, NG, C).astype(np.float32)


# revision 8
# speedup vs baseline: 8.7266x; 1.1901x over previous
"""Trainium2 Bass kernel for the fuzzy joint-membership layer.

Math (derived from the reference 2-qubit circuit, verified vs oracle):
  out[b, 2p,   c] = 0.5 + 0.5*cos(theta_c)*cos(x0) - 0.5*sin(theta_c)*sin(x0)*sin(x1)
  out[b, 2p+1, c] = 0.5 + 0.5*cos(x0)*cos(x1)
where x0 = xf[b, pair_idx[b,p,0]], x1 = xf[b, pair_idx[b,p,1]].

Sharding: pure data parallel, batch 4096 -> 8 cores x 512 rows.

Gather strategy: host-inverted local_scatter (per-partition independent
indices, vectorized Q7 loop in local RAM) instead of ap_gather (whose Q7 loop
costs ~24ns/idx and serialized the old kernel at ~83us per call):
  - host builds A_main[b, pix] = output slot of the FIRST occurrence of pixel
    `pix` in row b's 920-entry list (slot j = pair j's x0, 460+j = x1), -1 if
    unused; and Acomb[b, slot] = slot of the NEXT occurrence of the same
    pixel (chained generations), -1 if none.
  - device: G0 = local_scatter(xh, A_main); Gk = local_scatter(G{k-1}, Acomb)
    resolves duplicate gathers generation by generation (entries of other
    generations harmlessly scatter zeros). Gk have disjoint support.
  - m = max index multiplicity per row (data-dependent, ~6), baked at build.
x is converted to f16 on host (payload for the 2-byte scatter; output err
~1e-3, tolerance 2e-2).

GPSIMD shares its SBUF port with the Vector engine, so DVE ops stall ~5-17x
while a scatter streams. Hence the merge sum_k Gk runs on the idle PE
(identity matmuls accumulating into PSUM, exact, free f16->f32) and the
even-output base runs on ACT; DVE only does the range reduction, W/E
products, and one scalar_tensor_tensor per class.

Device per 128-row tile:
  - DMA in xh [128,3072] f16, A_main [128,3072] i16, Acomb [128,920] i16
  - gpsimd: m local_scatters (1 x 3072-scan + (m-1) x 920-scan)
  - PE: psum[h] = sum_k I @ Gk[:, h*460:...], h=0 (x0s) / 1 (x1s)
  - DVE range-reduction (magic round) + ACT Sin per half:
      cv_h = cos(vals_h), svN_h = -sin(vals_h)
  - DVE: W = svN0*svN1 = sin*sin, E = cv0*cv1
  - per class c (all writes contiguous, class-major device layout):
      even block = ACT(cv0*hct_c + 0.5), += W*nhst_c on DVE
      odd block = ACT(E*0.5 + 0.5)
  - DMA out [128, 9200]; host permutes class-major -> [B, 920, 10].
"""

import math
import numpy as np

B, PIX, NPAIR, C = 4096, 3072, 460, 10
NG = 2 * NPAIR          # 920 gathered values per row
OUTW = NG * C           # 9200
NCORES = 8
BS = B // NCORES        # 512 rows per core
TILES = BS // 128       # 4

_cache = {}


def _ensure_path():
    try:
        import concourse  # noqa: F401
    except ImportError:
        import sys
        sys.path.insert(0, "/opt/trn_rl_repo")


def build_nc(m, bs=BS):
    """m = max per-row index multiplicity -> 1 main scatter + (m-1) copy passes."""
    _ensure_path()
    from contextlib import ExitStack
    import concourse.tile as tile
    from concourse import bacc, mybir

    f32, f16, i16 = mybir.dt.float32, mybir.dt.float16, mybir.dt.int16
    Sin = mybir.ActivationFunctionType.Sin
    Copy = mybir.ActivationFunctionType.Copy
    mult = mybir.AluOpType.mult
    add = mybir.AluOpType.add
    sub_ = mybir.AluOpType.subtract
    maxop = mybir.AluOpType.max
    ntiles = bs // 128

    nc = bacc.Bacc("TRN2", target_bir_lowering=False, debug=False)
    xh_ext = nc.declare_dram_parameter("xh", [bs, PIX], f16, isOutput=False)
    am_ext = nc.declare_dram_parameter("amain", [bs, PIX], i16, isOutput=False)
    ac_ext = nc.declare_dram_parameter("acomb", [bs, NG], i16, isOutput=False)
    th_ext = nc.declare_dram_parameter("theta", [128, C], f32, isOutput=False)
    id_ext = nc.declare_dram_parameter("idmat", [128, 128], f16, isOutput=False)
    out_ext = nc.declare_dram_parameter("out", [bs, OUTW], f32, isOutput=True)

    PI, TWO_PI = math.pi, 2 * math.pi
    MAGIC, INV2PI = 1.5 * 2 ** 23, 1.0 / (2 * math.pi)

    with tile.TileContext(nc) as tc, ExitStack() as ctx:
        cpool = ctx.enter_context(tc.tile_pool(name="const", bufs=1))
        pihalf = cpool.tile([128, 1], f32)
        nc.vector.memset(pihalf[:], PI / 2)
        zerob = cpool.tile([128, 1], f32)
        nc.vector.memset(zerob[:], 0.0)
        idmat = cpool.tile([128, 128], f16)
        nc.sync.dma_start(out=idmat[:], in_=id_ext[:, :])

        xpool = ctx.enter_context(tc.tile_pool(name="xh", bufs=2))
        apool = ctx.enter_context(tc.tile_pool(name="amain", bufs=2))
        bpool = ctx.enter_context(tc.tile_pool(name="acomb", bufs=2))
        gpool = ctx.enter_context(tc.tile_pool(name="gens", bufs=2))
        ppool = ctx.enter_context(tc.tile_pool(name="psum", bufs=2, space="PSUM"))
        spool = ctx.enter_context(tc.tile_pool(name="trig", bufs=2))
        opool = ctx.enter_context(tc.tile_pool(name="ot", bufs=2))

        def trig(pool, src, width, tagp):
            """returns (cv, svN) = (cos(src), -sin(src)), width cols, f32.

            Range-reduce with round-to-nearest magic: n = (v/2pi + M) - M,
            -r = 2pi*n - v. Then -sin(v) = Sin(-r), cos(v) = Sin(pi/2 - |r|).
            """
            t1 = pool.tile([128, width], f32, tag=tagp + "t1")
            nc.vector.tensor_scalar(t1[:], src, INV2PI, MAGIC, mult, add)
            nc.vector.tensor_scalar(t1[:], t1[:], MAGIC, None, sub_)
            nc.vector.tensor_scalar(t1[:], t1[:], TWO_PI, None, mult)
            negr = pool.tile([128, width], f32, tag=tagp + "negr")
            nc.vector.tensor_tensor(negr[:], t1[:], src, sub_)
            nc.vector.tensor_scalar(t1[:], negr[:], -1.0, None, mult)
            nc.vector.tensor_tensor(t1[:], t1[:], negr[:], maxop)  # |r|
            cv = pool.tile([128, width], f32, tag=tagp + "cv")
            svN = pool.tile([128, width], f32, tag=tagp + "svN")
            nc.scalar.activation(svN[:], negr[:], Sin, bias=zerob[:, 0:1])
            nc.scalar.activation(cv[:], t1[:], Sin, bias=pihalf[:, 0:1], scale=-1.0)
            return cv, svN

        # theta coefficients: hct = 0.5*cos(theta), nhst = -0.5*sin(theta)
        th_sb = cpool.tile([128, C], f32)
        nc.sync.dma_start(out=th_sb[:], in_=th_ext[:, :])
        cvt, svNt = trig(cpool, th_sb[:], C, "th")
        hcoef = cpool.tile([128, 2 * C], f32)
        nc.vector.tensor_scalar(hcoef[:, 0:C], cvt[:], 0.5, None, mult)
        nc.vector.tensor_scalar(hcoef[:, C:2 * C], svNt[:], 0.5, None, mult)
        hct = hcoef[:, 0:C]        # 0.5*cos(theta)
        nhst = hcoef[:, C:2 * C]   # -0.5*sin(theta)

        for t in range(ntiles):
            rows = slice(t * 128, (t + 1) * 128)
            xh = xpool.tile([128, PIX], f16)
            nc.sync.dma_start(out=xh[:], in_=xh_ext[rows, :])
            am = apool.tile([128, PIX], i16)
            nc.sync.dma_start(out=am[:], in_=am_ext[rows, :])
            ac = bpool.tile([128, NG], i16)
            nc.sync.dma_start(out=ac[:], in_=ac_ext[rows, :])

            gens = [
                gpool.tile([128, NG], f16, tag=f"g{k}", name=f"g{k}")
                for k in range(m)
            ]
            nc.gpsimd.local_scatter(gens[0][:], xh[:], am[:], 128, NG, PIX)
            for k in range(1, m):
                nc.gpsimd.local_scatter(gens[k][:], gens[k - 1][:], ac[:], 128, NG, NG)

            # merge generations on the (otherwise idle) PE: psum_h = sum_k Gk
            ph = [
                ppool.tile([128, NPAIR], f32, tag=f"ph{h}", name=f"ph{h}")
                for h in range(2)
            ]
            for k in range(m):
                for h in range(2):
                    nc.tensor.matmul(
                        ph[h][:], idmat[:],
                        gens[k][:, h * NPAIR:(h + 1) * NPAIR],
                        start=(k == 0), stop=(k == m - 1),
                    )

            cv0, svN0 = trig(spool, ph[0][:], NPAIR, "h0")
            cv1, svN1 = trig(spool, ph[1][:], NPAIR, "h1")
            w = spool.tile([128, NPAIR], f32, tag="w")
            e = spool.tile([128, NPAIR], f32, tag="e")
            nc.vector.tensor_tensor(w[:], svN0[:], svN1[:], mult)
            nc.vector.tensor_tensor(e[:], cv0[:], cv1[:], mult)

            # class-major, parity-major device layout; host permutes back
            ot = opool.tile([128, OUTW], f32)
            for c in range(C):
                evs = ot[:, c * NG: c * NG + NPAIR]
                nc.scalar.activation(evs, cv0[:], Copy, bias=0.5, scale=hct[:, c:c + 1])
                nc.vector.scalar_tensor_tensor(evs, w[:], nhst[:, c:c + 1], evs, mult, add)
                ovs = ot[:, c * NG + NPAIR: (c + 1) * NG]
                nc.scalar.activation(ovs, e[:], Copy, bias=0.5, scale=0.5)
            nc.sync.dma_start(out=out_ext[rows, :], in_=ot[:])

    nc.compile()
    return nc


def _prep_indices(pair_idx):
    """Invert the per-row gather map into scatter index arrays.

    Slot layout: slot j = pair j's x0, slot 460+j = pair j's x1.
    Returns (A_main [B,3072] i16, Acomb [B,920] i16, m).
    """
    pi = np.asarray(pair_idx).reshape(B, NPAIR, 2)
    L = np.concatenate([pi[:, :, 0], pi[:, :, 1]], axis=1).astype(np.int64)
    order = np.argsort(L, axis=1, kind="stable")
    spix = np.take_along_axis(L, order, axis=1)
    first = np.ones_like(spix, dtype=bool)
    first[:, 1:] = spix[:, 1:] != spix[:, :-1]
    t = np.broadcast_to(np.arange(NG)[None, :], (B, NG))
    firstpos = np.maximum.accumulate(np.where(first, t, 0), axis=1)
    occ_sorted = t - firstpos
    m = int(occ_sorted.max()) + 1

    A_main = np.full((B, PIX), -1, np.int16)
    rr, cc = np.nonzero(occ_sorted == 0)
    A_main[rr, spix[rr, cc]] = order[rr, cc].astype(np.int16)

    Acomb = np.full((B, NG), -1, np.int16)
    nxt = np.zeros((B, NG), dtype=bool)
    nxt[:, :-1] = spix[:, 1:] == spix[:, :-1]
    rr2, cc2 = np.nonzero(nxt)
    Acomb[rr2, order[rr2, cc2]] = order[rr2, cc2 + 1].astype(np.int16)
    return A_main, Acomb, m


def _idmat_np():
    if "idmat" not in _cache:
        _cache["idmat"] = np.ascontiguousarray(np.eye(128, dtype=np.float16))
    return _cache["idmat"]


def _get_nc(m):
    key = ("nc", m)
    if key not in _cache:
        _cache[key] = build_nc(m)
    return _cache[key]


def kernel(x, pair_idx, theta):
    _ensure_path()
    from concourse.bass_utils import run_bass_kernel_spmd

    xh = np.ascontiguousarray(
        np.asarray(x, dtype=np.float32).reshape(B, PIX).astype(np.float16)
    )
    A_main, Acomb, m = _prep_indices(pair_idx)
    thb = np.ascontiguousarray(
        np.tile(np.asarray(theta, dtype=np.float32).reshape(1, C), (128, 1))
    )
    nc = _get_nc(m)
    in_maps = [
        {
            "xh": xh[k * BS:(k + 1) * BS],
            "amain": A_main[k * BS:(k + 1) * BS],
            "acomb": Acomb[k * BS:(k + 1) * BS],
            "theta": thb,
            "idmat": _idmat_np(),
        }
        for k in range(NCORES)
    ]
    res = run_bass_kernel_spmd(nc, in_maps, list(range(NCORES))).results
    out = np.concatenate([res[k]["out"] for k in range(NCORES)], axis=0)
    # device layout is [C, 2, NPAIR] per row -> [NPAIR, 2, C]
    out = out.reshape(B, C, 2, NPAIR).transpose(0, 3, 2, 1)
    return np.ascontiguousarray(out.reshape(B, NG, C), dtype=np.float32)


# revision 10
# speedup vs baseline: 8.8731x; 1.0168x over previous
"""Trainium2 Bass kernel for the fuzzy joint-membership layer.

Math (derived from the reference 2-qubit circuit, verified vs oracle):
  out[b, 2p,   c] = 0.5 + 0.5*cos(theta_c)*cos(x0) - 0.5*sin(theta_c)*sin(x0)*sin(x1)
  out[b, 2p+1, c] = 0.5 + 0.5*cos(x0)*cos(x1)
where x0 = xf[b, pair_idx[b,p,0]], x1 = xf[b, pair_idx[b,p,1]].

Sharding: pure data parallel, batch 4096 -> 8 cores x 512 rows.

Gather strategy: host-inverted local_scatter (per-partition independent
indices, vectorized Q7 loop in local RAM) instead of ap_gather (whose Q7 loop
costs ~24ns/idx and serialized the old kernel at ~83us per call):
  - host builds A_main[b, pix] = output slot of the FIRST occurrence of pixel
    `pix` in row b's 920-entry list (slot j = pair j's x0, 460+j = x1), -1 if
    unused; and Acomb[b, slot] = slot of the NEXT occurrence of the same
    pixel (chained generations), -1 if none.
  - device: G0 = local_scatter(xh, A_main); Gk = local_scatter(G{k-1}, Acomb)
    resolves duplicate gathers generation by generation (entries of other
    generations harmlessly scatter zeros). Gk have disjoint support.
  - m = max index multiplicity per row (data-dependent, ~6), baked at build.
x is converted to f16 on host (payload for the 2-byte scatter; output err
~1e-3, tolerance 2e-2).

GPSIMD shares its SBUF port with the Vector engine, so DVE ops stall ~5-17x
while a scatter streams. Hence the merge sum_k Gk runs on the idle PE
(identity matmuls accumulating into PSUM, exact, free f16->f32) and the
even-output base runs on ACT; DVE only does the range reduction, W/E
products, and one scalar_tensor_tensor per class.

Device per 128-row tile:
  - DMA in xh [128,3072] f16, A_main [128,3072] i16, Acomb [128,920] i16
  - gpsimd: m local_scatters (1 x 3072-scan + (m-1) x 920-scan)
  - PE: psum[h] = sum_k I @ Gk[:, h*460:...], h=0 (x0s) / 1 (x1s)
  - DVE range-reduction (magic round) + ACT Sin per half:
      cv_h = cos(vals_h), svN_h = -sin(vals_h)
  - DVE: W = svN0*svN1 = sin*sin, E = cv0*cv1
  - per class c (all writes contiguous, class-major device layout):
      even block = ACT(cv0*hct_c + 0.5), += W*nhst_c on DVE
      odd block = ACT(E*0.5 + 0.5)
  - DMA out [128, 9200]; host permutes class-major -> [B, 920, 10].
"""

import math
import numpy as np

B, PIX, NPAIR, C = 4096, 3072, 460, 10
NG = 2 * NPAIR          # 920 gathered values per row
OUTW = NG * C           # 9200
NCORES = 8
BS = B // NCORES        # 512 rows per core
TILES = BS // 128       # 4

_cache = {}


def _ensure_path():
    try:
        import concourse  # noqa: F401
    except ImportError:
        import sys
        sys.path.insert(0, "/opt/trn_rl_repo")


def build_nc(m, bs=BS):
    """m = max per-row index multiplicity -> 1 main scatter + (m-1) copy passes."""
    _ensure_path()
    from contextlib import ExitStack
    import concourse.tile as tile
    from concourse import bacc, mybir

    f32, f16, i16 = mybir.dt.float32, mybir.dt.float16, mybir.dt.int16
    Sin = mybir.ActivationFunctionType.Sin
    Copy = mybir.ActivationFunctionType.Copy
    mult = mybir.AluOpType.mult
    add = mybir.AluOpType.add
    sub_ = mybir.AluOpType.subtract
    maxop = mybir.AluOpType.max
    ntiles = bs // 128

    nc = bacc.Bacc("TRN2", target_bir_lowering=False, debug=False)
    xh_ext = nc.declare_dram_parameter("xh", [bs, PIX], f16, isOutput=False)
    am_ext = nc.declare_dram_parameter("amain", [bs, PIX], i16, isOutput=False)
    ac_ext = nc.declare_dram_parameter("acomb", [bs, NG], i16, isOutput=False)
    th_ext = nc.declare_dram_parameter("theta", [128, C], f32, isOutput=False)
    id_ext = nc.declare_dram_parameter("idmat", [128, 128], f16, isOutput=False)
    out_ext = nc.declare_dram_parameter("out", [bs, OUTW], f32, isOutput=True)

    PI, TWO_PI = math.pi, 2 * math.pi
    MAGIC, INV2PI = 1.5 * 2 ** 23, 1.0 / (2 * math.pi)

    with tile.TileContext(nc) as tc, ExitStack() as ctx:
        cpool = ctx.enter_context(tc.tile_pool(name="const", bufs=1))
        pihalf = cpool.tile([128, 1], f32)
        nc.vector.memset(pihalf[:], PI / 2)
        zerob = cpool.tile([128, 1], f32)
        nc.vector.memset(zerob[:], 0.0)
        idmat = cpool.tile([128, 128], f16)
        nc.sync.dma_start(out=idmat[:], in_=id_ext[:, :])

        xpool = ctx.enter_context(tc.tile_pool(name="xh", bufs=2))
        apool = ctx.enter_context(tc.tile_pool(name="amain", bufs=2))
        bpool = ctx.enter_context(tc.tile_pool(name="acomb", bufs=2))
        gpool = ctx.enter_context(tc.tile_pool(name="gens", bufs=2))
        ppool = ctx.enter_context(tc.tile_pool(name="psum", bufs=2, space="PSUM"))
        spool = ctx.enter_context(tc.tile_pool(name="trig", bufs=2))
        opool = ctx.enter_context(tc.tile_pool(name="ot", bufs=2))

        def trig(pool, src, width, tagp):
            """returns (cv, svN) = (cos(src), -sin(src)), width cols, f32.

            Range-reduce with round-to-nearest magic: n = (v/2pi + M) - M,
            -r = 2pi*n - v. Then -sin(v) = Sin(-r), cos(v) = Sin(pi/2 - |r|).
            """
            t1 = pool.tile([128, width], f32, tag=tagp + "t1")
            nc.vector.tensor_scalar(t1[:], src, INV2PI, MAGIC, mult, add)
            nc.vector.tensor_scalar(t1[:], t1[:], MAGIC, None, sub_)
            nc.vector.tensor_scalar(t1[:], t1[:], TWO_PI, None, mult)
            negr = pool.tile([128, width], f32, tag=tagp + "negr")
            nc.vector.tensor_tensor(negr[:], t1[:], src, sub_)
            nc.vector.tensor_scalar(t1[:], negr[:], -1.0, None, mult)
            nc.vector.tensor_tensor(t1[:], t1[:], negr[:], maxop)  # |r|
            cv = pool.tile([128, width], f32, tag=tagp + "cv")
            svN = pool.tile([128, width], f32, tag=tagp + "svN")
            nc.scalar.activation(svN[:], negr[:], Sin, bias=zerob[:, 0:1])
            nc.scalar.activation(cv[:], t1[:], Sin, bias=pihalf[:, 0:1], scale=-1.0)
            return cv, svN

        # theta coefficients: hct = 0.5*cos(theta), nhst = -0.5*sin(theta)
        th_sb = cpool.tile([128, C], f32)
        nc.sync.dma_start(out=th_sb[:], in_=th_ext[:, :])
        cvt, svNt = trig(cpool, th_sb[:], C, "th")
        hcoef = cpool.tile([128, 2 * C], f32)
        nc.vector.tensor_scalar(hcoef[:, 0:C], cvt[:], 0.5, None, mult)
        nc.vector.tensor_scalar(hcoef[:, C:2 * C], svNt[:], 0.5, None, mult)
        hct = hcoef[:, 0:C]        # 0.5*cos(theta)
        nhst = hcoef[:, C:2 * C]   # -0.5*sin(theta)

        for t in range(ntiles):
            rows = slice(t * 128, (t + 1) * 128)
            xh = xpool.tile([128, PIX], f16)
            nc.sync.dma_start(out=xh[:], in_=xh_ext[rows, :])
            am = apool.tile([128, PIX], i16)
            nc.sync.dma_start(out=am[:], in_=am_ext[rows, :])
            ac = bpool.tile([128, NG], i16)
            nc.sync.dma_start(out=ac[:], in_=ac_ext[rows, :])

            gens = [
                gpool.tile([128, NG], f16, tag=f"g{k}", name=f"g{k}")
                for k in range(m)
            ]
            nc.gpsimd.local_scatter(gens[0][:], xh[:], am[:], 128, NG, PIX)
            for k in range(1, m):
                nc.gpsimd.local_scatter(gens[k][:], gens[k - 1][:], ac[:], 128, NG, NG)

            # merge generations on the (otherwise idle) PE: psum_h = sum_k Gk
            ph = [
                ppool.tile([128, NPAIR], f32, tag=f"ph{h}", name=f"ph{h}")
                for h in range(2)
            ]
            for k in range(m):
                for h in range(2):
                    nc.tensor.matmul(
                        ph[h][:], idmat[:],
                        gens[k][:, h * NPAIR:(h + 1) * NPAIR],
                        start=(k == 0), stop=(k == m - 1),
                    )

            cv0, svN0 = trig(spool, ph[0][:], NPAIR, "h0")
            cv1, svN1 = trig(spool, ph[1][:], NPAIR, "h1")
            w = spool.tile([128, NPAIR], f32, tag="w")
            e = spool.tile([128, NPAIR], f32, tag="e")
            nc.vector.tensor_tensor(w[:], svN0[:], svN1[:], mult)
            nc.vector.tensor_tensor(e[:], cv0[:], cv1[:], mult)

            # parity-major, class-major device layout; host permutes back.
            # Odd outputs (ACT only) complete first and their half DMAs out
            # while the even half still computes -- shrinks the tile tail.
            HALF = C * NPAIR  # 4600
            ot = opool.tile([128, OUTW], f32)
            for c in range(C):
                ovs = ot[:, HALF + c * NPAIR: HALF + (c + 1) * NPAIR]
                nc.scalar.activation(ovs, e[:], Copy, bias=0.5, scale=0.5)
            nc.sync.dma_start(out=out_ext[rows, HALF:OUTW], in_=ot[:, HALF:OUTW])
            for c in range(C):
                evs = ot[:, c * NPAIR: (c + 1) * NPAIR]
                nc.scalar.activation(evs, cv0[:], Copy, bias=0.5, scale=hct[:, c:c + 1])
                nc.vector.scalar_tensor_tensor(evs, w[:], nhst[:, c:c + 1], evs, mult, add)
            nc.sync.dma_start(out=out_ext[rows, 0:HALF], in_=ot[:, 0:HALF])

    nc.compile()
    return nc


def _prep_indices(pair_idx):
    """Invert the per-row gather map into scatter index arrays.

    Slot layout: slot j = pair j's x0, slot 460+j = pair j's x1.
    Returns (A_main [B,3072] i16, Acomb [B,920] i16, m).
    """
    pi = np.asarray(pair_idx).reshape(B, NPAIR, 2)
    L = np.concatenate([pi[:, :, 0], pi[:, :, 1]], axis=1).astype(np.int64)
    order = np.argsort(L, axis=1, kind="stable")
    spix = np.take_along_axis(L, order, axis=1)
    first = np.ones_like(spix, dtype=bool)
    first[:, 1:] = spix[:, 1:] != spix[:, :-1]
    t = np.broadcast_to(np.arange(NG)[None, :], (B, NG))
    firstpos = np.maximum.accumulate(np.where(first, t, 0), axis=1)
    occ_sorted = t - firstpos
    m = int(occ_sorted.max()) + 1

    A_main = np.full((B, PIX), -1, np.int16)
    rr, cc = np.nonzero(occ_sorted == 0)
    A_main[rr, spix[rr, cc]] = order[rr, cc].astype(np.int16)

    Acomb = np.full((B, NG), -1, np.int16)
    nxt = np.zeros((B, NG), dtype=bool)
    nxt[:, :-1] = spix[:, 1:] == spix[:, :-1]
    rr2, cc2 = np.nonzero(nxt)
    Acomb[rr2, order[rr2, cc2]] = order[rr2, cc2 + 1].astype(np.int16)
    return A_main, Acomb, m


def _idmat_np():
    if "idmat" not in _cache:
        _cache["idmat"] = np.ascontiguousarray(np.eye(128, dtype=np.float16))
    return _cache["idmat"]


def _get_nc(m):
    key = ("nc", m)
    if key not in _cache:
        _cache[key] = build_nc(m)
    return _cache[key]


def kernel(x, pair_idx, theta):
    _ensure_path()
    from concourse.bass_utils import run_bass_kernel_spmd

    xh = np.ascontiguousarray(
        np.asarray(x, dtype=np.float32).reshape(B, PIX).astype(np.float16)
    )
    A_main, Acomb, m = _prep_indices(pair_idx)
    thb = np.ascontiguousarray(
        np.tile(np.asarray(theta, dtype=np.float32).reshape(1, C), (128, 1))
    )
    nc = _get_nc(m)
    in_maps = [
        {
            "xh": xh[k * BS:(k + 1) * BS],
            "amain": A_main[k * BS:(k + 1) * BS],
            "acomb": Acomb[k * BS:(k + 1) * BS],
            "theta": thb,
            "idmat": _idmat_np(),
        }
        for k in range(NCORES)
    ]
    res = run_bass_kernel_spmd(nc, in_maps, list(range(NCORES))).results
    out = np.concatenate([res[k]["out"] for k in range(NCORES)], axis=0)
    # device layout is [2, C, NPAIR] per row (parity, class, pair) ->
    # [NPAIR, 2, C]; parity 0 = even outputs, 1 = odd
    out = out.reshape(B, 2, C, NPAIR).transpose(0, 3, 1, 2)
    return np.ascontiguousarray(out.reshape(B, NG, C), dtype=np.float32)
